# revision 53
# baseline (speedup 1.0000x reference)
"""Trainium2 Bass kernel for nn_EnhancedStateEncoder (6-layer dense transformer).

Strategy: data-parallel over batch across 8 NeuronCores (2 batches/core).
 - Embedding + sinusoidal pos-emb folded on host (cheap gather).
 - Attention heads processed in mixed pairs, one "scalar-path" head and one
   "fused-path" head each, on distinct PE-array tiles: QK matmuls (K=32) on
   two distinct 32-row PE bands; PV matmuls (M=33) on the two 64-column PE
   groups, the pair accumulating into partitions [0:33]/[64:97] of one PSUM
   tile so evacuation covers both heads at full partition occupancy.
 - softmax numerator exp(SCALE*qk + bias) split across engines per head:
   scalar-path heads use the exact Exp activation (ScalarE) then multiply by
   a resident bf16 exp(bias) table (Vector/GpSimd alternating); fused-path
   heads run ONE Vector op - a Schraudolph exp with the bias folded in:
   int16(A*qk + [B + A'*bias]) whose bits are the bf16 result (~3% max err,
   validated 5.2e-3 end-to-end). The bias-sum table is precomputed host-side
   as int16 so the per-element add is exact.
 - Two-stage software pipeline across layers: each layer's batch-0 attention
   absorbs the previous layer's deferred MLP tail (batch-1 token chunks) and
   LN1 feeds; batch-1 attention overlaps this layer's first MLP chunks and
   the next layer's LN1 - all four engines stay loaded.
 - MLP second matmul writes a 4-chunk PSUM quad; its bias rides two
   start=True matmuls emitted FIRST (has_written clears are bank-wide per
   partition, so per-slice accumulation groups would wipe bank-neighbors).
 - LayerNorm via bn_stats/bn_aggr; rsqrt as exp(-0.5*ln(var+eps)).
 - LN2's affine is folded into the MLP's first matmul on the host.
"""

import math
import os
from contextlib import ExitStack

import numpy as np
import ml_dtypes

import concourse.bass as bass
import concourse.mybir as mybir
import concourse.tile as tile
from concourse.bass_utils import run_bass_kernel_spmd
from concourse.masks import make_identity

F32 = mybir.dt.float32
BF16 = mybir.dt.bfloat16
I16 = mybir.dt.int16

B, S, D, H, HD, L, H2 = 16, 1024, 256, 8, 32, 6, 1024
NC = 8            # cores
BL = B // NC      # batches per core = 2
T = BL * S        # tokens per core = 2048
NCH = T // 128    # 128-token chunks per core = 16
SCALE = 1.0 / math.sqrt(HD)
LN_EPS = 1e-5
GRID = 32

# Schraudolph exp-as-bf16-bits: i16 = round(SCH_A*x + SCH_B); bits(i16) ~ exp(x)
SCH_A = (2.0 ** 7) / math.log(2.0) * SCALE   # folds the 1/sqrt(hd) score scale
SCH_B = 127.0 * 2.0 ** 7 - 0.0430357 * 2.0 ** 7

_cache = {}




def _alibi_tables():
    """Transposed bias tables, split by head path:
    ebT[p, hs, jc, i] = exp(bias[h, i, jc*128+p]) bf16 for heads {0,1,4,5};
    lebT[p, hd, jc, i] = round(SCH_B + 2^7/ln2 * bias) int16 for {2,3,6,7}.
    """
    if "ebT" in _cache:
        return _cache["ebT"], _cache["lebT"]
    xs, ys = np.meshgrid(np.arange(GRID), np.arange(GRID), indexing="ij")
    xf = xs.reshape(-1).astype(np.float32)
    yf = ys.reshape(-1).astype(np.float32)
    dist = np.abs(xf[:, None] - xf[None, :]) + np.abs(yf[:, None] - yf[None, :])
    i = np.arange(H, dtype=np.float32)
    sl = -(2.0 ** (-(1.0 + i)))
    sr = -(2.0 ** (-(0.5 + i)))
    eb = np.empty((128, 4, S // 128, S), dtype=ml_dtypes.bfloat16)
    leb = np.empty((128, 4, S // 128, S), dtype=np.int16)
    triu = np.triu(np.ones((S, S), np.bool_))  # j >= i
    for h in range(H):
        b = np.where(triu, sr[h] * dist, sl[h] * dist)  # [i, j] log-bias
        slot = (h % 2) + 2 * (h // 4)
        if (h // 2) % 2 == 0:
            bT = np.ascontiguousarray(np.exp(b).T)  # [j, i]
            eb[:, slot] = bT.reshape(S // 128, 128, S).transpose(1, 0, 2).astype(
                ml_dtypes.bfloat16)
        else:
            lb = np.rint(SCH_B + (2.0 ** 7 / np.log(2.0)) * b).astype(np.int16)
            lbT = np.ascontiguousarray(lb.T)
            leb[:, slot] = lbT.reshape(S // 128, 128, S).transpose(1, 0, 2)
    _cache["ebT"], _cache["lebT"] = eb, leb
    return eb, leb


def _pos_table():
    if "pos" in _cache:
        return _cache["pos"]
    inv_freq = 1.0 / (10000.0 ** (np.arange(0, D, 2, dtype=np.float32) / D))
    t = np.arange(S, dtype=np.float32)
    sinusoid = t[:, None] * inv_freq[None, :]
    _cache["pos"] = np.concatenate(
        [np.sin(sinusoid), np.cos(sinusoid)], axis=-1
    ).astype(np.float32)
    return _cache["pos"]


def _build_bass():
    if "nc" in _cache:
        return _cache["nc"]
    nc = bass.Bass()
    io = {}
    io["x0"] = nc.dram_tensor("x0", [128, NCH, D], F32, kind="ExternalInput")
    io["ebT"] = nc.dram_tensor("ebT", [128, 4, S // 128, S], BF16, kind="ExternalInput")
    io["lebT"] = nc.dram_tensor("lebT", [128, 4, S // 128, S], I16, kind="ExternalInput")
    io["w1h"] = nc.dram_tensor("w1h", [L, 128, D // 128, H2], BF16, kind="ExternalInput")
    io["b1h"] = nc.dram_tensor("b1h", [L, 128, H2 // 128], F32, kind="ExternalInput")
    io["w2h"] = nc.dram_tensor("w2h", [L, 128, H2 // 128, D], BF16, kind="ExternalInput")
    io["b2h"] = nc.dram_tensor("b2h", [L, 1, D], BF16, kind="ExternalInput")
    io["ln1w"] = nc.dram_tensor("ln1w", [L, 128, D], BF16, kind="ExternalInput")
    io["ln1b"] = nc.dram_tensor("ln1b", [L, 128, D], BF16, kind="ExternalInput")
    io["lnfw"] = nc.dram_tensor("lnfw", [128, D], BF16, kind="ExternalInput")
    io["lnfb"] = nc.dram_tensor("lnfb", [128, D], BF16, kind="ExternalInput")
    y = nc.dram_tensor("y", [128, NCH, D], F32, kind="ExternalOutput")

    with tile.TileContext(nc) as tc, ExitStack() as ctx:
        _emit(ctx, tc, io, y)

    _split_multi_waits(nc)
    _cache["nc"] = nc
    return nc


def _split_multi_waits(nc):
    """walrus codegen on this image only supports ONE sync-wait per TPB
    engine-instruction descriptor. Move excess waits onto sequencer NoOps
    inserted immediately before the instruction (same engine queue)."""
    nsplit = 0
    skip = ("InstNoOp", "InstEventSemaphore")
    for func in nc.m.functions:
        for bb in func.blocks:
            insts = list(bb.instructions)
            out = []
            for inst in insts:
                si = inst.sync_info
                if (si is not None and si.on_wait and len(si.on_wait) > 1
                        and type(inst).__name__ not in skip):
                    for w in list(si.on_wait[:-1]):
                        nop = mybir.InstNoOp(
                            name=f"WSPLIT-{nsplit}", ins=[], outs=[])
                        nop.engine = inst.engine
                        nop.sync_info = mybir.SyncInfo(
                            on_wait=[w], on_update=[])
                        out.append(nop)
                        nsplit += 1
                    si.on_wait = [si.on_wait[-1]]
                out.append(inst)
            if nsplit:
                bb.instructions = out
    return nsplit


def _emit(ctx, tc, io, y):
    nc = tc.nc
    singles = ctx.enter_context(tc.tile_pool(name="singles", bufs=1))
    lnp = ctx.enter_context(tc.tile_pool(name="lnp", bufs=1))
    wp = ctx.enter_context(tc.tile_pool(name="wp", bufs=1))
    xp = ctx.enter_context(tc.tile_pool(name="xp", bufs=2))
    sp = ctx.enter_context(tc.tile_pool(name="sp", bufs=4))
    ep = ctx.enter_context(tc.tile_pool(name="ep", bufs=11))
    tmp_p = ctx.enter_context(tc.tile_pool(name="tmp", bufs=2))
    # PSUM: ps pool (2 bufs x [128,1024]f32 = 4 banks) for QK scores and MLP
    # first-matmul tiles; po pool (2 bufs x [128,1024]f32 = 4 banks) for PV
    # accumulation (head pair at partitions 0:33 / 64:97), ptt transposes and
    # MLP second-matmul quads.
    ps_p = ctx.enter_context(tc.tile_pool(name="ps", bufs=3, space="PSUM"))
    po_p = ctx.enter_context(tc.tile_pool(name="po", bufs=1, space="PSUM"))

    # ---- resident tensors ----
    x_sb = singles.tile([128, NCH, D], F32)
    for g in range(4):
        nc.sync.dma_start(out=x_sb[:, 4 * g:4 * g + 4, :],
                          in_=io["x0"][:, 4 * g:4 * g + 4, :])
    # bias factors, split by head path: exp(bias) bf16 for the scalar-exp
    # heads {0,1,4,5}; Schraudolph-domain int16 log-bias for the fused-DVE
    # heads {2,3,6,7} (i16 = round(SCH_B + 2^7/ln2 * bias)).
    eb_sb = singles.tile([128, 4, S // 128, S], BF16)
    for h in range(4):
        nc.sync.dma_start(out=eb_sb[:, h], in_=io["ebT"][:, h])
    leb_sb = singles.tile([128, 4, S // 128, S], I16)
    for h in range(4):
        nc.sync.dma_start(out=leb_sb[:, h], in_=io["lebT"][:, h])
    id_bf16 = singles.tile([128, 128], BF16)
    make_identity(nc, id_bf16)
    ones_col = singles.tile([1, 128], BF16)
    nc.vector.memset(ones_col, 1.0)
    eps_t = singles.tile([128, 1], F32)
    nc.vector.memset(eps_t, LN_EPS)
    absorb_scratch = singles.tile([128, 16], F32)
    absorb_n = [0]

    def absorb(ap):
        # DVE wait absorber: DVE-struct instructions support only one sync
        # wait on this codegen, so soak the DMA-completion wait into a copy.
        k = absorb_n[0] % 16
        absorb_n[0] += 1
        nc.vector.tensor_copy(out=absorb_scratch[:, k:k + 1],
                              in_=ap[0:128, 0:1])
    v_aug = singles.tile([128, NCH, H, 34], BF16)
    nc.vector.memset(v_aug, 1.0)
    # xnT: [128 (4 heads x 32 dims), half, T] transposed LN'd activations
    xnT = singles.tile([128, 2, T], BF16)
    hT = singles.tile([128, 8, 512], BF16)

    def ln_stats(c, mv):
        st = sp.tile([128, 6], F32, tag="st")
        nc.vector.bn_stats(out=st, in_=x_sb[:, c, :])
        nc.vector.bn_aggr(out=mv[:, c, :], in_=st)

    def ln_rsqrt(mv, rs):
        nc.scalar.activation(
            out=rs, in_=mv[:, :, 1],
            func=mybir.ActivationFunctionType.Ln, bias=eps_t, scale=1.0,
        )
        nc.scalar.activation(
            out=rs, in_=rs,
            func=mybir.ActivationFunctionType.Exp, scale=-0.5,
        )

    def ln_rsqrt_half(mv, rs, b):
        sl = slice(b * 8, (b + 1) * 8)
        nc.scalar.activation(
            out=rs[:, sl], in_=mv[:, sl, 1],
            func=mybir.ActivationFunctionType.Ln, bias=eps_t, scale=1.0,
        )
        nc.scalar.activation(
            out=rs[:, sl], in_=rs[:, sl],
            func=mybir.ActivationFunctionType.Exp, scale=-0.5,
        )

    def ln_norm(c, mv, rs, affine):
        """normalized bf16 chunk of x_sb[:, c, :]"""
        xn = xp.tile([128, D], BF16, tag="xn")
        if affine is not None:
            w_sb, b_sb = affine
            nc.vector.tensor_scalar(
                out=xn, in0=x_sb[:, c, :],
                scalar1=mv[:, c, 0:1], scalar2=rs[:, c:c + 1],
                op0=mybir.AluOpType.subtract, op1=mybir.AluOpType.mult,
            )
            nc.vector.tensor_mul(out=xn, in0=xn, in1=w_sb)
            nc.vector.tensor_add(out=xn, in0=xn, in1=b_sb)
        else:
            nc.vector.tensor_scalar(
                out=xn, in0=x_sb[:, c, :],
                scalar1=mv[:, c, 0:1], scalar2=rs[:, c:c + 1],
                op0=mybir.AluOpType.subtract, op1=mybir.AluOpType.mult,
            )
        return xn

    def feed_xn(xn, c, with_vaug=True):
        pt = ps_p.tile([128, 2, 128], BF16, tag="ps")
        for half in range(2):
            nc.tensor.transpose(
                pt[:, half, :], xn[:, half * 128:(half + 1) * 128], id_bf16
            )
        nc.vector.tensor_copy(
            out=xnT[:, :, c * 128:(c + 1) * 128], in_=pt
        )
        if with_vaug:
            nc.gpsimd.tensor_copy(
                out=v_aug[:, c, :, 0:HD],
                in_=xn.rearrange("p (h d) -> p h d", h=H),
            )

    mvA = sp.tile([128, NCH, 2], F32, tag="mvA")
    for c in range(NCH):
        ln_stats(c, mvA)

    carry = [None]
    wh = [None]
    for l in range(L):
        # per-layer params
        if l == 0:
            ln1w_sb = lnp.tile([128, D], BF16, tag="ln1w")
            nc.sync.dma_start(out=ln1w_sb, in_=io["ln1w"][l])
            absorb(ln1w_sb)
            ln1b_sb = lnp.tile([128, D], BF16, tag="ln1b")
            nc.sync.dma_start(out=ln1b_sb, in_=io["ln1b"][l])
            absorb(ln1b_sb)
        # ---- phase A: LN1. For l>0 this was already emitted at the tail
        # of the previous layer, pipelined under its MLP. ----
        if l == 0:
            rsA = sp.tile([128, NCH], F32, tag="rsA")
            ln_rsqrt(mvA, rsA)
            for c in range(NCH):
                feed_xn(ln_norm(c, mvA, rsA, (ln1w_sb, ln1b_sb)), c)

        # ---- phase B: attention, head pairs on concurrent PE tiles ----
        def qk_it(h, b, jc, ps, it):
            hh, hp = h // 4, (h % 4) * HD
            ktile = xnT[hp:hp + HD, hh,
                        b * S + jc * 128: b * S + (jc + 1) * 128]
            qtile = xnT[hp:hp + HD, hh,
                        b * S + it * 512: b * S + (it + 1) * 512]
            nc.tensor.matmul(
                ps[:, it * 512:(it + 1) * 512],
                lhsT=ktile, rhs=qtile, start=True, stop=True,
                tile_position=(hp, 0),
            )

        # Engine-balanced softmax numerator. Heads 0,1,4,5 store exp(bias)
        # as bf16 (exact multiply); heads 2,3,6,7 store the Schraudolph-
        # domain log-bias as int16, usable either fused into one Vector op
        # (exp+bias in a single instruction) or bitcast to bf16 (~exp(bias))
        # as a plain multiply operand. Per-tile engine pattern tuned so
        # Scalar/Vector/GpSimd land near-equal load.
        def exp_mul(ps, h, jc):
            et = ep.tile([128, S], BF16, tag="et")
            slot = (h % 2) + 2 * (h // 4)
            if (h // 2) % 2 == 0:
                # exact exp on Scalar, bias multiply on GpSimd/Vector
                nc.scalar.activation(
                    out=et, in_=ps,
                    func=mybir.ActivationFunctionType.Exp, scale=SCALE,
                )
                eng = nc.gpsimd if jc % 2 == 0 else nc.vector
                eng.tensor_tensor(
                    out=et, in0=et, in1=eb_sb[:, slot, jc, :],
                    op=mybir.AluOpType.mult,
                )
            else:
                # fused Schraudolph exp + log-bias add on Vector
                nc.vector.scalar_tensor_tensor(
                    out=et.bitcast(I16), in0=ps, scalar=SCH_A,
                    in1=leb_sb[:, slot, jc, :],
                    op0=mybir.AluOpType.mult, op1=mybir.AluOpType.add,
                )
            return et

        def pv(b, h0, h1, jc, po, et_0, et_1):
            for it in range(2):
                sl = slice(it * 512, (it + 1) * 512)
                nc.tensor.matmul(
                    po[0:33, sl],
                    lhsT=v_aug[:, b * 8 + jc, h0, 0:33],
                    rhs=et_0[:, sl],
                    start=(jc == 0), stop=(jc == S // 128 - 1),
                    tile_position=(0, 0),
                )
                nc.tensor.matmul(
                    po[64:97, sl],
                    lhsT=v_aug[:, b * 8 + jc, h1, 0:33],
                    rhs=et_1[:, sl],
                    start=(jc == 0), stop=(jc == S // 128 - 1),
                    tile_position=(0, 64),
                )

        def po_cast(po):
            # evacuate both heads in one Scalar cast, freeing the po buffer
            ot = ep.tile([128, S], BF16, tag="et")
            nc.scalar.activation(
                out=ot, in_=po,
                func=mybir.ActivationFunctionType.Copy,
            )
            return ot

        def po_post(b, h0, h1, ot):
            # one K=97 transpose per block covers both heads (dead rows
            # 33:64 in between); divide; residual add on GpSimd.
            ptt = po_p.tile([128, 8, 98], BF16, tag="po")
            for ic in range(S // 128):
                nc.tensor.transpose(
                    ptt[:, ic, 0:97],
                    ot[0:97, ic * 128:(ic + 1) * 128],
                    id_bf16[0:97, 0:97],
                )
            rt = sp.tile([128, 2, 8, 1], F32, tag="rt")
            for e in range(2):
                nc.vector.reciprocal(out=rt[:, e],
                                     in_=ptt[:, :, 64 * e + 32:64 * e + 33])
            tmp = tmp_p.tile([128, 2, 8, HD], BF16, tag="tmp")
            for e in range(2):
                nc.vector.tensor_tensor(
                    out=tmp[:, e], in0=ptt[:, :, 64 * e:64 * e + HD],
                    in1=rt[:, e].broadcast_to([128, 8, HD]),
                    op=mybir.AluOpType.mult,
                )
            for e, h in enumerate((h0, h1)):
                xs = x_sb[:, b * 8:(b + 1) * 8, h * HD:(h + 1) * HD]
                nc.gpsimd.tensor_add(out=xs, in0=xs, in1=tmp[:, e])

        # Pairs mix one Scalar-path head with one fused-Vector-path head so
        # every pair loads Scalar, Vector and GpSimd evenly; the two heads
        # sit on distinct 32-row PE bands and distinct PV column groups.
        PAIRS = ((0, 2), (1, 3), (4, 6), (5, 7))
        mvC = sp.tile([128, NCH, 2], F32, tag="mvC")
        rsC = sp.tile([128, NCH], F32, tag="rsC")
        pending_post = [None]

        def attn_pair(b, pi):
            h0, h1 = PAIRS[pi]
            po = None
            for halfj in range(2):
                jcs = range(4 * halfj, 4 * halfj + 4)
                ets = {}
                for jc in jcs:
                    ps_0 = ps_p.tile([128, S], F32, tag="ps")
                    ps_1 = ps_p.tile([128, S], F32, tag="ps")
                    # interleave the two heads' matmuls: adjacent
                    # instructions on distinct 32-row PE bands overlap
                    for it in range(2):
                        qk_it(h0, b, jc, ps_0, it)
                        qk_it(h1, b, jc, ps_1, it)
                    ets[jc] = (exp_mul(ps_0, h0, jc),
                               exp_mul(ps_1, h1, jc))
                if halfj == 0:
                    # previous pair's transposes/divide/residual run here
                    # so its PSUM cast never stalls the PE pipeline
                    if pending_post[0] is not None:
                        po_post(*pending_post[0])
                        pending_post[0] = None
                    po = po_p.tile([128, S], F32, tag="po")
                for jc in jcs:
                    pv(b, h0, h1, jc, po, *ets[jc])
            ot = po_cast(po)
            if pi == len(PAIRS) - 1:
                po_post(b, h0, h1, ot)
                # batch residual stream final: LN2 stats + rsqrt ride along
                for c in range(b * 8, b * 8 + 8):
                    ln_stats(c, mvC)
                ln_rsqrt_half(mvC, rsC, b)
            else:
                pending_post[0] = (b, h0, h1, ot)

        def mlp_tt(tt, mvA, w, mvC=mvC, rsC=rsC):
            w1_sb, b1_sb, w2_sb, b2_sb = w
            for c in range(4 * tt, 4 * tt + 4):
                feed_xn(ln_norm(c, mvC, rsC, None), c, with_vaug=False)
            for hbp in range(4):  # pairs of h2-blocks
                pm = ps_p.tile([128, S], F32, tag="ps")
                for sub in range(2):
                    hb = hbp * 2 + sub
                    for k in range(D // 128):
                        nc.tensor.matmul(
                            pm[:, sub * 512:(sub + 1) * 512],
                            lhsT=w1_sb[:, k, hb * 128:(hb + 1) * 128],
                            rhs=xnT[:, k, tt * 512:(tt + 1) * 512],
                            start=(k == 0), stop=(k == D // 128 - 1),
                        )
                for sub in range(2):
                    hb = hbp * 2 + sub
                    nc.scalar.activation(
                        out=hT[:, hb, :],
                        in_=pm[:, sub * 512:(sub + 1) * 512],
                        func=mybir.ActivationFunctionType.Gelu,
                        bias=b1_sb[:, hb:hb + 1],
                    )
            # bias first: each start=True matmul covers one full PSUM bank
            # (2 t2-slices), setting has_written everywhere so the w2 matmuls
            # below accumulate. (start clears has_written bank-wide per
            # partition, so per-slice groups would wipe their bank-neighbor.)
            pm2 = po_p.tile([128, 4, D], F32, tag="po")
            b2b = b2_sb.rearrange("a (r d) -> a r d", r=1).broadcast_to([1, 2, D])
            for t2h in range(2):
                nc.tensor.matmul(
                    pm2[:, 2 * t2h:2 * t2h + 2, :], lhsT=ones_col, rhs=b2b,
                    start=True, stop=False,
                )
            for t2 in range(4):
                for hb in range(H2 // 128):
                    nc.tensor.matmul(
                        pm2[:, t2, :],
                        lhsT=hT[:, hb, t2 * 128:(t2 + 1) * 128],
                        rhs=w2_sb[:, hb, :],
                        start=False, stop=(hb == H2 // 128 - 1),
                    )
            nc.vector.tensor_add(
                out=x_sb[:, 4 * tt:4 * tt + 4, :],
                in0=x_sb[:, 4 * tt:4 * tt + 4, :], in1=pm2,
            )
            for t2 in range(4):
                # stats for the next LN (LN1 of l+1, or the final LN)
                ln_stats(tt * 4 + t2, mvA)

        # Two-stage software pipeline across layers: this layer's attn(b0)
        # section absorbs the PREVIOUS layer's deferred MLP tail (tts 2,3 =
        # batch-1 tokens, independent of b0 attention) and the LN1(b1) feeds
        # for THIS layer; attn(b1) overlaps this layer's MLP tts 0,1 and the
        # next layer's LN1(b0) feeds.
        attn_pair(0, 0)
        attn_pair(0, 1)
        if carry[0]:
            carry[0][0]()        # mlp_tt(2, l-1)
        attn_pair(0, 2)
        if carry[0]:
            carry[0][1]()        # mlp_tt(3, l-1)
            carry[0][2]()        # LN1(l, batch-1 half)
            carry[0] = None
        # this layer's MLP weights load once the previous layer's carried
        # MLP matmuls (their last readers) have been emitted
        w1_sb = wp.tile([128, D // 128, H2], BF16, tag="w1")
        nc.sync.dma_start(out=w1_sb, in_=io["w1h"][l])
        b1_sb = wp.tile([128, H2 // 128], F32, tag="b1")
        nc.sync.dma_start(out=b1_sb, in_=io["b1h"][l])
        w2_sb = wp.tile([128, H2 // 128, D], BF16, tag="w2")
        nc.sync.dma_start(out=w2_sb, in_=io["w2h"][l])
        b2_sb = wp.tile([1, D], BF16, tag="b2")
        nc.sync.dma_start(out=b2_sb, in_=io["b2h"][l])
        wh[0] = (w1_sb, b1_sb, w2_sb, b2_sb)
        attn_pair(0, 3)

        mvA = sp.tile([128, NCH, 2], F32, tag="mvA")
        attn_pair(1, 0)
        mlp_tt(0, mvA, wh[0])
        attn_pair(1, 1)
        mlp_tt(1, mvA, wh[0])
        attn_pair(1, 2)
        if l < L - 1:
            # next layer's LN1 (batch-0 half) pipelined under attention
            ln1w_nx = lnp.tile([128, D], BF16, tag="ln1w")
            nc.sync.dma_start(out=ln1w_nx, in_=io["ln1w"][l + 1])
            absorb(ln1w_nx)
            ln1b_nx = lnp.tile([128, D], BF16, tag="ln1b")
            nc.sync.dma_start(out=ln1b_nx, in_=io["ln1b"][l + 1])
            absorb(ln1b_nx)
            rsA = sp.tile([128, NCH], F32, tag="rsA")
            ln_rsqrt_half(mvA, rsA, 0)
            for c in range(8):
                feed_xn(ln_norm(c, mvA, rsA, (ln1w_nx, ln1b_nx)), c)
        attn_pair(1, 3)

        def _carry2(mvA=mvA, w=wh[0], mvC=mvC, rsC=rsC):
            mlp_tt(2, mvA, w, mvC, rsC)

        def _carry3(mvA=mvA, w=wh[0], mvC=mvC, rsC=rsC):
            mlp_tt(3, mvA, w, mvC, rsC)

        if l < L - 1:
            def _carry_ln1(mvA=mvA, rsA=rsA, lw=ln1w_nx, lb=ln1b_nx):
                ln_rsqrt_half(mvA, rsA, 1)
                for c in range(8, NCH):
                    feed_xn(ln_norm(c, mvA, rsA, (lw, lb)), c)
        else:
            _carry_ln1 = None
        carry[0] = (_carry2, _carry3, _carry_ln1)

    # flush the last layer's deferred MLP tail
    carry[0][0]()
    carry[0][1]()
    carry[0] = None

    # ---- final LN (in place, f32) ----
    lnfw_sb = lnp.tile([128, D], BF16, tag="ln1w")
    nc.sync.dma_start(out=lnfw_sb, in_=io["lnfw"][:])
    absorb(lnfw_sb)
    lnfb_sb = lnp.tile([128, D], BF16, tag="ln1b")
    nc.sync.dma_start(out=lnfb_sb, in_=io["lnfb"][:])
    absorb(lnfb_sb)

    rsF = sp.tile([128, NCH], F32, tag="rsA")
    ln_rsqrt(mvA, rsF)
    for c in range(NCH):
        xf = xp.tile([128, D], F32, tag="xf")
        nc.vector.tensor_scalar(
            out=xf, in0=x_sb[:, c, :],
            scalar1=mvA[:, c, 0:1], scalar2=rsF[:, c:c + 1],
            op0=mybir.AluOpType.subtract, op1=mybir.AluOpType.mult,
        )
        nc.vector.tensor_mul(out=xf, in0=xf, in1=lnfw_sb)
        nc.vector.tensor_add(out=xf, in0=xf, in1=lnfb_sb)
        nc.sync.dma_start(out=y[:, c, :], in_=xf)


def _install_ntff_hook():
    """Wire antenv.axon_hooks NTFF profiling via libaxon ctypes (dev only)."""
    if _cache.get("hook_done"):
        return
    _cache["hook_done"] = True
    try:
        import types
        import sys
        try:
            from antenv.axon_hooks import set_axon_ntff_profile_hook  # noqa
        except ImportError:
            import antenv
            mod = types.ModuleType("antenv.axon_hooks")
            holder = [None]
            mod.set_axon_ntff_profile_hook = lambda h: holder.__setitem__(0, h)
            mod.get_axon_ntff_profile_hook = lambda: holder[0]
            sys.modules["antenv.axon_hooks"] = mod
            antenv.axon_hooks = mod
            from trn_agent_boot.trn_boot import _ntff_profile_via_ctypes
            mod.set_axon_ntff_profile_hook(
                _ntff_profile_via_ctypes("/opt/axon/libaxon_pjrt.so"))
    except Exception as e:  # fail-soft: tracing degrades, run still works
        print("ntff hook install failed:", e)


def kernel(tokens, pos_ids, emb_table, input_weight, position_weight,
           ln1_w, ln1_b, ln2_w, ln2_b, w1, b1, w2, b2, lnf_w, lnf_b):
    tokens = np.asarray(tokens)
    pos_ids = np.asarray(pos_ids)
    emb_table = np.asarray(emb_table, dtype=np.float32)
    x0 = (np.float32(np.asarray(input_weight).reshape(-1)[0])
          * emb_table[tokens]
          + np.float32(np.asarray(position_weight).reshape(-1)[0])
          * _pos_table()[np.asarray(pos_ids)][None]).astype(np.float32)

    w1 = np.asarray(w1, np.float32)
    b1 = np.asarray(b1, np.float32)
    w2 = np.asarray(w2, np.float32)
    b2 = np.asarray(b2, np.float32)
    ln2_w = np.asarray(ln2_w, np.float32)
    ln2_b = np.asarray(ln2_b, np.float32)
    # fold LN2 affine into MLP weights
    w1eff = ln2_w[:, :, None] * w1                     # [L, D, H2]
    b1eff = b1 + np.einsum("ld,ldh->lh", ln2_b, w1)    # [L, H2]
    w1h = np.ascontiguousarray(
        w1eff.reshape(L, D // 128, 128, H2).transpose(0, 2, 1, 3)
    ).astype(ml_dtypes.bfloat16)
    w2h = np.ascontiguousarray(
        w2.reshape(L, H2 // 128, 128, D).transpose(0, 2, 1, 3)
    ).astype(ml_dtypes.bfloat16)

    nc = _build_bass()
    base = {
        "ebT": _alibi_tables()[0],
        "lebT": _alibi_tables()[1],
        "w1h": w1h,
        "b1h": np.ascontiguousarray(
            b1eff.reshape(L, H2 // 128, 128).transpose(0, 2, 1)),
        "w2h": w2h,
        "b2h": np.ascontiguousarray(
            np.asarray(b2)[:, None, :]).astype(ml_dtypes.bfloat16),
        "ln1w": np.ascontiguousarray(np.broadcast_to(
            np.asarray(ln1_w)[:, None, :], (L, 128, D))
        ).astype(ml_dtypes.bfloat16),
        "ln1b": np.ascontiguousarray(np.broadcast_to(
            np.asarray(ln1_b)[:, None, :], (L, 128, D))
        ).astype(ml_dtypes.bfloat16),
        "lnfw": np.ascontiguousarray(np.broadcast_to(
            np.asarray(lnf_w)[None, :], (128, D))
        ).astype(ml_dtypes.bfloat16),
        "lnfb": np.ascontiguousarray(np.broadcast_to(
            np.asarray(lnf_b)[None, :], (128, D))
        ).astype(ml_dtypes.bfloat16),
    }
    in_maps = []
    for core in range(NC):
        xc = x0[core * BL:(core + 1) * BL].reshape(T, D)
        xh = np.ascontiguousarray(
            xc.reshape(NCH, 128, D).transpose(1, 0, 2))
        m = dict(base)
        m["x0"] = xh
        in_maps.append(m)

    trace = os.environ.get("KERNEL_TRACE", "0") == "1"
    if trace:
        _install_ntff_hook()
    res = run_bass_kernel_spmd(
        nc, in_maps, core_ids=list(range(NC)), trace=trace,
        trace_cores=[0] if trace else None,
    )
    if trace and res.exec_time_ns is not None:
        print(f"HW exec time: {res.exec_time_ns} ns")
        if res.instructions_and_trace is not None:
            print("trace:", res.instructions_and_trace[1])

    out = np.empty((B, S, D), np.float32)
    for core in range(NC):
        yh = res.results[core]["y"]  # [128, NCH, D]
        yc = yh.transpose(1, 0, 2).reshape(BL, S, D)
        out[core * BL:(core + 1) * BL] = yc
    return out
